# revision 1
# baseline (speedup 1.0000x reference)
"""Cross-attention block (q from z_hsi, k/v from z_msi, softmax over 6400
pixels, residual + gamma) on 8 Trainium2 NeuronCores.

Sharding: the (batch=2, N=6400) query-pixel space is split into 8 shards of
1600 pixels (4 shards per batch element). Each core computes its shard's
attention output against the full key/value set of its batch element; the
host slices inputs and concatenates outputs (no device collectives).

Per-core math, all matmuls in float32r (TF32-like, full PE rate):
  K  = Wk @ zm + bk            [128, 6400]   (lhsT = Wk^T, rhs = zm)
  Q  = Wq @ zq + bq            [128, 1600]
  VT = (Wv @ zm)^T             [6400, 128]   (lhsT = zm tile, rhs = Wv^T;
                                              bv folded in later via d)
  per 512-wide query block i:
    ET[j, i] = K^T Q           (j on partitions -> softmax needs no
                                attention transpose)
    P = exp(ET)                (ACT, PSUM->SBUF, no max subtraction:
                                |E| < ~20 so exp is fp32-safe)
    d[i]  = ones^T P           (denominator, accumulated over j tiles)
    PV[o,i] = VT_tile^T P      (accumulated over j tiles)
    out = PV * (gamma/d) + (zq + gamma*bv)
The gamma/d broadcast across partitions is one K=1 fp32 matmul.
"""
import sys

sys.path.insert(0, "/opt/trn_rl_repo")

import numpy as np
import concourse.bass as bass  # noqa: F401  (import keeps bass registered)
import concourse.tile as tile
from concourse import bacc, mybir
from concourse.bass_utils import run_bass_kernel_spmd

B, CH, CM, CO = 2, 128, 64, 128
H = W = 80
N = H * W            # 6400 key/value pixels per batch element
NCORES = 8
NI = (B * N) // NCORES   # 1600 query pixels per core
JT = N // 128            # 50 key tiles
F32 = mybir.dt.float32
F32R = mybir.dt.float32r

I_BLOCKS = [(0, 512), (512, 512), (1024, 512), (1536, 64)]


def _build():
    nc = bacc.Bacc(None, target_bir_lowering=False)
    zq = nc.declare_dram_parameter("zq", [CH, NI], F32R, isOutput=False)
    zm = nc.declare_dram_parameter("zm", [CM, N], F32R, isOutput=False)
    wqT = nc.declare_dram_parameter("wqT", [CH, CO], F32R, isOutput=False)
    wkT = nc.declare_dram_parameter("wkT", [CM, CO], F32R, isOutput=False)
    wvT = nc.declare_dram_parameter("wvT", [CM, CO], F32R, isOutput=False)
    bq = nc.declare_dram_parameter("bq", [CO, 1], F32, isOutput=False)
    bk = nc.declare_dram_parameter("bk", [CO, 1], F32, isOutput=False)
    gbv = nc.declare_dram_parameter("gbv", [CO, 1], F32, isOutput=False)
    gcol = nc.declare_dram_parameter("gcol", [1, CO], F32, isOutput=False)
    ones = nc.declare_dram_parameter("ones", [128, 1], F32R, isOutput=False)
    out = nc.declare_dram_parameter("out", [CO, NI], F32, isOutput=True)

    with tile.TileContext(nc) as tc:
        with (
            tc.tile_pool(name="big", bufs=1) as big,
            tc.tile_pool(name="expp", bufs=3) as expp,
            tc.tile_pool(name="work", bufs=2) as work,
            tc.tile_pool(name="pse", bufs=2, space="PSUM") as pse,
            tc.tile_pool(name="pspv", bufs=2, space="PSUM") as pspv,
        ):
            zm_sb = big.tile([CM, N], F32R)
            nc.sync.dma_start(zm_sb[:], zm[:])
            zq_sb = big.tile([CH, NI], F32R)
            nc.sync.dma_start(zq_sb[:], zq[:])
            wq_sb = big.tile([CH, CO], F32R)
            nc.sync.dma_start(wq_sb[:], wqT[:])
            wk_sb = big.tile([CM, CO], F32R)
            nc.sync.dma_start(wk_sb[:], wkT[:])
            wv_sb = big.tile([CM, CO], F32R)
            nc.sync.dma_start(wv_sb[:], wvT[:])
            bq_sb = big.tile([CO, 1], F32)
            nc.sync.dma_start(bq_sb[:], bq[:])
            bk_sb = big.tile([CO, 1], F32)
            nc.sync.dma_start(bk_sb[:], bk[:])
            gbv_sb = big.tile([CO, 1], F32)
            nc.sync.dma_start(gbv_sb[:], gbv[:])
            gcol_sb = big.tile([1, CO], F32)
            nc.sync.dma_start(gcol_sb[:], gcol[:])
            ones_sb = big.tile([128, 1], F32R)
            nc.sync.dma_start(ones_sb[:], ones[:])

            # residual (+ folded gamma*bv), exact fp32 bits of z_hsi
            zqp = big.tile([CH, NI], F32)
            nc.vector.tensor_scalar_add(zqp[:], zq_sb[:].bitcast(F32), gbv_sb[:])

            K_sb = big.tile([CO, N], F32R)
            Q_sb = big.tile([CO, NI], F32R)
            VT_sb = big.tile([128, JT * CO], F32R)

            # K projection: K[o, j] = sum_c Wk[o,c] zm[c,j] + bk[o]
            for c0 in range(0, N, 512):
                cs = min(512, N - c0)
                pk = pse.tile([128, 1024], F32, tag="e")
                nc.tensor.matmul(pk[:, :cs], wk_sb[:], zm_sb[:, c0:c0 + cs],
                                 start=True, stop=True)
                nc.vector.tensor_scalar_add(K_sb[:, c0:c0 + cs], pk[:, :cs], bk_sb[:])

            # Q projection
            for c0 in range(0, NI, 512):
                cs = min(512, NI - c0)
                pq = pse.tile([128, 1024], F32, tag="e")
                nc.tensor.matmul(pq[:, :cs], wq_sb[:], zq_sb[:, c0:c0 + cs],
                                 start=True, stop=True)
                nc.vector.tensor_scalar_add(Q_sb[:, c0:c0 + cs], pq[:, :cs], bq_sb[:])

            # VT tiles: VT[j, o] = sum_c zm[c, j] Wv[o, c]   (no bias: folded)
            for g0 in range(0, JT, 4):
                nq = min(4, JT - g0)
                pvt = pse.tile([128, 1024], F32, tag="e")
                for jj in range(nq):
                    j0 = (g0 + jj) * 128
                    nc.tensor.matmul(pvt[:, jj * 128:(jj + 1) * 128],
                                     zm_sb[:, j0:j0 + 128], wv_sb[:],
                                     start=True, stop=True)
                nc.vector.tensor_copy(VT_sb[:, g0 * 128:(g0 + nq) * 128],
                                      pvt[:, :nq * 128])

            # main attention loop over query blocks
            for i0, ibs in I_BLOCKS:
                pv = pspv.tile([128, 512], F32, tag="pv")
                dsum = pspv.tile([128, 512], F32, tag="d")
                for g in range(JT // 2):
                    e2 = pse.tile([128, 1024], F32, tag="e")
                    for h in (0, 1):
                        jt = 2 * g + h
                        nc.tensor.matmul(
                            e2[:, h * 512:h * 512 + ibs],
                            K_sb[:, jt * 128:(jt + 1) * 128],
                            Q_sb[:, i0:i0 + ibs],
                            start=True, stop=True)
                    p2 = expp.tile([128, 1024], F32R, tag="p")
                    if ibs == 512:
                        nc.scalar.activation(p2[:], e2[:],
                                             mybir.ActivationFunctionType.Exp)
                    else:
                        for h in (0, 1):
                            nc.scalar.activation(
                                p2[:, h * 512:h * 512 + ibs],
                                e2[:, h * 512:h * 512 + ibs],
                                mybir.ActivationFunctionType.Exp)
                    for h in (0, 1):
                        jt = 2 * g + h
                        nc.tensor.matmul(
                            dsum[:1, :ibs], ones_sb[:],
                            p2[:, h * 512:h * 512 + ibs],
                            start=(jt == 0), stop=(jt == JT - 1),
                            skip_group_check=True)
                        nc.tensor.matmul(
                            pv[:, :ibs],
                            VT_sb[:, jt * 128:(jt + 1) * 128],
                            p2[:, h * 512:h * 512 + ibs],
                            start=(jt == 0), stop=(jt == JT - 1),
                            skip_group_check=True)
                # normalize: out = PV * (gamma/d) + zqp
                d_inv = work.tile([1, 512], F32, tag="dinv")
                nc.vector.reciprocal(d_inv[:, :ibs], dsum[:1, :ibs])
                b_ps = pspv.tile([128, 512], F32, tag="d")
                nc.tensor.matmul(b_ps[:, :ibs], gcol_sb[:], d_inv[:, :ibs],
                                 start=True, stop=True)
                b_sb = work.tile([128, 512], F32, tag="bsb")
                nc.scalar.copy(b_sb[:, :ibs], b_ps[:, :ibs])
                t_sb = work.tile([128, 512], F32, tag="tsb")
                nc.vector.tensor_mul(t_sb[:, :ibs], pv[:, :ibs], b_sb[:, :ibs])
                o_sb = work.tile([128, 512], F32, tag="osb")
                nc.vector.tensor_add(o_sb[:, :ibs], t_sb[:, :ibs],
                                     zqp[:, i0:i0 + ibs])
                nc.sync.dma_start(out[:, i0:i0 + ibs], o_sb[:, :ibs])

    nc.finalize()
    return nc


_cached_nc = None


def kernel(z_hsi, z_msi, Wq, bq, Wk, bk, Wv, bv, gamma):
    global _cached_nc
    if _cached_nc is None:
        _cached_nc = _build()
    nc = _cached_nc

    z_hsi = np.asarray(z_hsi, dtype=np.float32).reshape(B, CH, N)
    z_msi = np.ascontiguousarray(np.asarray(z_msi, dtype=np.float32).reshape(B, CM, N))
    wqT = np.ascontiguousarray(np.asarray(Wq, dtype=np.float32).T)
    wkT = np.ascontiguousarray(np.asarray(Wk, dtype=np.float32).T)
    wvT = np.ascontiguousarray(np.asarray(Wv, dtype=np.float32).T)
    g = float(np.asarray(gamma, dtype=np.float32).reshape(-1)[0])
    bq_c = np.ascontiguousarray(np.asarray(bq, np.float32).reshape(CO, 1))
    bk_c = np.ascontiguousarray(np.asarray(bk, np.float32).reshape(CO, 1))
    gbv = np.ascontiguousarray((g * np.asarray(bv, np.float32)).reshape(CO, 1))
    gcol = np.full((1, CO), g, dtype=np.float32)
    ones = np.ones((128, 1), dtype=np.float32)

    shards_per_b = NCORES // B
    in_maps = []
    for c in range(NCORES):
        b, s = c // shards_per_b, (c % shards_per_b) * NI
        in_maps.append({
            "zq": np.ascontiguousarray(z_hsi[b][:, s:s + NI]),
            "zm": z_msi[b],
            "wqT": wqT, "wkT": wkT, "wvT": wvT,
            "bq": bq_c, "bk": bk_c, "gbv": gbv, "gcol": gcol, "ones": ones,
        })

    res = run_bass_kernel_spmd(nc, in_maps, core_ids=list(range(NCORES)))

    out = np.empty((B, CH, N), dtype=np.float32)
    for c in range(NCORES):
        b, s = c // shards_per_b, (c % shards_per_b) * NI
        out[b][:, s:s + NI] = res.results[c]["out"]
    return out.reshape(B, CH, H, W)


# revision 5
# speedup vs baseline: 1.1110x; 1.1110x over previous
"""Cross-attention block (q from z_hsi, k/v from z_msi, softmax over 6400
pixels, residual + gamma) on 8 Trainium2 NeuronCores.

Sharding: the (batch=2, N=6400) query-pixel space is split into 8 shards of
1600 pixels (4 shards per batch element). Each core computes its shard's
attention output against the full key/value set of its batch element; the
host slices inputs and concatenates outputs (no device collectives).

Per-core math, all matmuls in float32r (TF32-like, full PE rate):
  K  = Wk @ zm + bk            [128, 6400]   (lhsT = Wk^T, rhs = zm)
  Q  = Wq @ zq + bq            [128, 1600]
  VT = (Wv @ zm)^T             [6400, 128]   (lhsT = zm tile, rhs = Wv^T;
                                              bv folded in later via d)
  per 512-wide query block i:
    ET[j, i] = K^T Q           (j on partitions -> softmax needs no
                                attention transpose)
    P = exp(ET)                (ACT, PSUM->SBUF, no max subtraction:
                                |E| < ~20 so exp is fp32-safe)
    d[i]  = ones^T P           (denominator, accumulated over j tiles)
    PV[o,i] = VT_tile^T P      (accumulated over j tiles)
    out = PV * (gamma/d) + (zq + gamma*bv)
The gamma/d broadcast across partitions is one K=1 fp32 matmul.
"""
import sys

sys.path.insert(0, "/opt/trn_rl_repo")

import numpy as np
import concourse.bass as bass  # noqa: F401  (import keeps bass registered)
import concourse.tile as tile
from concourse import bacc, mybir
from concourse.bass_utils import run_bass_kernel_spmd

B, CH, CM, CO = 2, 128, 64, 128
H = W = 80
N = H * W            # 6400 key/value pixels per batch element
NCORES = 8
NI = (B * N) // NCORES   # 1600 query pixels per core
JT = N // 128            # 50 key tiles
F32 = mybir.dt.float32
F32R = mybir.dt.float32r

I_BLOCKS = [(0, 512), (512, 512), (1024, 512), (1536, 64)]


def _build(repeat=1):
    """repeat>1 wraps the whole per-core compute in an on-device For_i loop;
    used only by the perf harness to measure HW time via wall-clock slope."""
    nc = bacc.Bacc(None, target_bir_lowering=False)
    zq = nc.declare_dram_parameter("zq", [CH, NI], F32R, isOutput=False)
    zm = nc.declare_dram_parameter("zm", [CM, N], F32R, isOutput=False)
    wqT = nc.declare_dram_parameter("wqT", [CH, CO], F32R, isOutput=False)
    wkT = nc.declare_dram_parameter("wkT", [CM, CO], F32R, isOutput=False)
    wvT = nc.declare_dram_parameter("wvT", [CM, CO], F32R, isOutput=False)
    bq = nc.declare_dram_parameter("bq", [CO, 1], F32, isOutput=False)
    bk = nc.declare_dram_parameter("bk", [CO, 1], F32, isOutput=False)
    gbv = nc.declare_dram_parameter("gbv", [CO, 1], F32, isOutput=False)
    gcol = nc.declare_dram_parameter("gcol", [1, CO], F32, isOutput=False)
    ones = nc.declare_dram_parameter("ones", [128, 1], F32R, isOutput=False)
    out = nc.declare_dram_parameter("out", [CO, NI], F32, isOutput=True)

    with tile.TileContext(nc) as tc:
        with (
            tc.tile_pool(name="big", bufs=1) as big,
            tc.tile_pool(name="expp", bufs=3) as expp,
            tc.tile_pool(name="work", bufs=2) as work,
            tc.tile_pool(name="pse", bufs=2, space="PSUM") as pse,
            tc.tile_pool(name="pspv", bufs=2, space="PSUM") as pspv,
        ):
            zm_sb = big.tile([CM, N], F32R)
            nc.sync.dma_start(zm_sb[:], zm[:])
            zq_sb = big.tile([CH, NI], F32R)
            nc.sync.dma_start(zq_sb[:], zq[:])
            wq_sb = big.tile([CH, CO], F32R)
            nc.sync.dma_start(wq_sb[:], wqT[:])
            wk_sb = big.tile([CM, CO], F32R)
            nc.sync.dma_start(wk_sb[:], wkT[:])
            wv_sb = big.tile([CM, CO], F32R)
            nc.sync.dma_start(wv_sb[:], wvT[:])
            bq_sb = big.tile([CO, 1], F32)
            nc.sync.dma_start(bq_sb[:], bq[:])
            bk_sb = big.tile([CO, 1], F32)
            nc.sync.dma_start(bk_sb[:], bk[:])
            gbv_sb = big.tile([CO, 1], F32)
            nc.sync.dma_start(gbv_sb[:], gbv[:])
            gcol_sb = big.tile([1, CO], F32)
            nc.sync.dma_start(gcol_sb[:], gcol[:])
            ones_sb = big.tile([128, 1], F32R)
            nc.sync.dma_start(ones_sb[:], ones[:])

            from contextlib import ExitStack, nullcontext
            rep_ctx = tc.For_i(0, repeat, 1) if repeat > 1 else nullcontext()
            with rep_ctx:
                _emit_body(nc, tc, big, expp, work, pse, pspv,
                           zm_sb, zq_sb, wq_sb, wk_sb, wv_sb,
                           bq_sb, bk_sb, gbv_sb, gcol_sb, ones_sb, out)

    nc.finalize()
    return nc


def _emit_body(nc, tc, big, expp, work, pse, pspv,
               zm_sb, zq_sb, wq_sb, wk_sb, wv_sb,
               bq_sb, bk_sb, gbv_sb, gcol_sb, ones_sb, out):
    if True:
        if True:
            # residual (+ folded gamma*bv), exact fp32 bits of z_hsi
            zqp = big.tile([CH, NI], F32)
            nc.vector.tensor_scalar_add(zqp[:], zq_sb[:].bitcast(F32), gbv_sb[:])

            K_sb = big.tile([CO, N], F32R)
            Q_sb = big.tile([CO, NI], F32R)
            VT_sb = big.tile([128, JT * CO], F32R)

            # K projection: K[o, j] = sum_c Wk[o,c] zm[c,j] + bk[o]
            for c0 in range(0, N, 512):
                cs = min(512, N - c0)
                pk = pse.tile([128, 1024], F32, tag="e")
                nc.tensor.matmul(pk[:, :cs], wk_sb[:], zm_sb[:, c0:c0 + cs],
                                 start=True, stop=True)
                nc.vector.tensor_scalar_add(K_sb[:, c0:c0 + cs], pk[:, :cs], bk_sb[:])

            # Q projection
            for c0 in range(0, NI, 512):
                cs = min(512, NI - c0)
                pq = pse.tile([128, 1024], F32, tag="e")
                nc.tensor.matmul(pq[:, :cs], wq_sb[:], zq_sb[:, c0:c0 + cs],
                                 start=True, stop=True)
                nc.vector.tensor_scalar_add(Q_sb[:, c0:c0 + cs], pq[:, :cs], bq_sb[:])

            # VT tiles: VT[j, o] = sum_c zm[c, j] Wv[o, c]   (no bias: folded)
            for g0 in range(0, JT, 4):
                nq = min(4, JT - g0)
                pvt = pse.tile([128, 1024], F32, tag="e")
                for jj in range(nq):
                    j0 = (g0 + jj) * 128
                    nc.tensor.matmul(pvt[:, jj * 128:(jj + 1) * 128],
                                     zm_sb[:, j0:j0 + 128], wv_sb[:],
                                     start=True, stop=True)
                nc.vector.tensor_copy(VT_sb[:, g0 * 128:(g0 + nq) * 128],
                                      pvt[:, :nq * 128])

            # main attention loop over query blocks.  d/PV matmuls are
            # emitted one pair behind E/exp so the PE never waits on the
            # exp it just enabled (software pipeline, PE<->ACT overlap).
            for i0, ibs in I_BLOCKS:
                pv = pspv.tile([128, 512], F32, tag="pv")
                dsum = pspv.tile([128, 512], F32, tag="d")

                def emit_dpv(p2_prev, g):
                    for h in (0, 1):
                        jt = 2 * g + h
                        nc.tensor.matmul(
                            dsum[:1, :ibs], ones_sb[:],
                            p2_prev[:, h * 512:h * 512 + ibs],
                            start=(jt == 0), stop=(jt == JT - 1),
                            skip_group_check=True)
                        nc.tensor.matmul(
                            pv[:, :ibs],
                            VT_sb[:, jt * 128:(jt + 1) * 128],
                            p2_prev[:, h * 512:h * 512 + ibs],
                            start=(jt == 0), stop=(jt == JT - 1),
                            skip_group_check=True)

                prev = None
                for g in range(JT // 2):
                    e2 = pse.tile([128, 1024], F32, tag="e")
                    for h in (0, 1):
                        jt = 2 * g + h
                        nc.tensor.matmul(
                            e2[:, h * 512:h * 512 + ibs],
                            K_sb[:, jt * 128:(jt + 1) * 128],
                            Q_sb[:, i0:i0 + ibs],
                            start=True, stop=True)
                    p2 = expp.tile([128, 1024], F32R, tag="p")
                    if ibs == 512:
                        nc.scalar.activation(p2[:], e2[:],
                                             mybir.ActivationFunctionType.Exp)
                    else:
                        for h in (0, 1):
                            nc.scalar.activation(
                                p2[:, h * 512:h * 512 + ibs],
                                e2[:, h * 512:h * 512 + ibs],
                                mybir.ActivationFunctionType.Exp)
                    if prev is not None:
                        emit_dpv(*prev)
                    prev = (p2, g)
                emit_dpv(*prev)

                # normalize: out = PV * (gamma/d) + zqp
                d_inv = work.tile([1, 512], F32, tag="dinv")
                nc.vector.reciprocal(d_inv[:, :ibs], dsum[:1, :ibs])
                b_ps = pspv.tile([128, 512], F32, tag="d")
                nc.tensor.matmul(b_ps[:, :ibs], gcol_sb[:], d_inv[:, :ibs],
                                 start=True, stop=True)
                b_sb = work.tile([128, 512], F32, tag="bsb")
                nc.vector.tensor_copy(b_sb[:, :ibs], b_ps[:, :ibs])
                t_sb = work.tile([128, 512], F32, tag="tsb")
                nc.vector.tensor_mul(t_sb[:, :ibs], pv[:, :ibs], b_sb[:, :ibs])
                o_sb = work.tile([128, 512], F32, tag="osb")
                nc.vector.tensor_add(o_sb[:, :ibs], t_sb[:, :ibs],
                                     zqp[:, i0:i0 + ibs])
                nc.sync.dma_start(out[:, i0:i0 + ibs], o_sb[:, :ibs])


_cached_nc = None


def kernel(z_hsi, z_msi, Wq, bq, Wk, bk, Wv, bv, gamma):
    global _cached_nc
    if _cached_nc is None:
        _cached_nc = _build()
    nc = _cached_nc

    z_hsi = np.asarray(z_hsi, dtype=np.float32).reshape(B, CH, N)
    z_msi = np.ascontiguousarray(np.asarray(z_msi, dtype=np.float32).reshape(B, CM, N))
    wqT = np.ascontiguousarray(np.asarray(Wq, dtype=np.float32).T)
    wkT = np.ascontiguousarray(np.asarray(Wk, dtype=np.float32).T)
    wvT = np.ascontiguousarray(np.asarray(Wv, dtype=np.float32).T)
    g = float(np.asarray(gamma, dtype=np.float32).reshape(-1)[0])
    bq_c = np.ascontiguousarray(np.asarray(bq, np.float32).reshape(CO, 1))
    bk_c = np.ascontiguousarray(np.asarray(bk, np.float32).reshape(CO, 1))
    gbv = np.ascontiguousarray((g * np.asarray(bv, np.float32)).reshape(CO, 1))
    gcol = np.full((1, CO), g, dtype=np.float32)
    ones = np.ones((128, 1), dtype=np.float32)

    shards_per_b = NCORES // B
    in_maps = []
    for c in range(NCORES):
        b, s = c // shards_per_b, (c % shards_per_b) * NI
        in_maps.append({
            "zq": np.ascontiguousarray(z_hsi[b][:, s:s + NI]),
            "zm": z_msi[b],
            "wqT": wqT, "wkT": wkT, "wvT": wvT,
            "bq": bq_c, "bk": bk_c, "gbv": gbv, "gcol": gcol, "ones": ones,
        })

    res = run_bass_kernel_spmd(nc, in_maps, core_ids=list(range(NCORES)))

    out = np.empty((B, CH, N), dtype=np.float32)
    for c in range(NCORES):
        b, s = c // shards_per_b, (c % shards_per_b) * NI
        out[b][:, s:s + NI] = res.results[c]["out"]
    return out.reshape(B, CH, H, W)
